# revision 1
# baseline (speedup 1.0000x reference)
"""Causal self-attention kernel for 8 Trainium2 NeuronCores.

Problem: B=2, T=2048, D=2048, H=16, Dh=128, fp32 in/out.
  qkv = x @ Wqkv + bqkv ; per-head causal attention ; out = att @ Wout + bout

Sharding (tensor parallel over heads + AllToAll before out_proj):
  Core c owns heads {2c, 2c+1}. Each core computes Q^T/K^T (head-dim on
  partitions) and V (token-dim on partitions) for all 4096 tokens via the
  QKV projection with its 768-column shard of Wqkv, runs causal attention
  locally (scores computed transposed: S^T[k,q], softmax reduction over k
  via an all-ones matmul which also broadcasts the denominator), and
  produces att^T per batch. Four AllToAlls (one per half-batch of tokens)
  redistribute head-sharded -> token-sharded; core c projects its 128-token
  slices with the full Wout (resident in SBUF).

Schedule: flash-style interleave. Attention group (hl, qc) is emitted as
soon as proj chunks covering tokens <= (qc+1)*512 land, so AllToAlls fire
mid-phase; batch-0's out-projection runs inside batch-1's proj/attention
phase, leaving only batch-1's out-projection in the tail.

All matmul operands are bf16 (fp32 PSUM accumulation); softmax denominators
use reciprocal_approx_fast (fp32, ~18-bit).
"""

import numpy as np
import ml_dtypes

import concourse.bass as bass
import concourse.mybir as mybir
import concourse.tile as tile
from concourse import bacc
from concourse.bass_utils import run_bass_kernel_spmd

B, T, D, H, Dh = 2, 2048, 2048, 16, 128
NT = B * T                  # 4096 tokens total
W = 8                       # cores
HL = H // W                 # 2 heads per core
CQKV = 3 * HL * Dh          # 768 qkv columns per core
KO = D // 128               # 16 contraction subtiles
TC = 256                    # proj token chunk
NTC_B = T // TC             # 8 chunks per batch
QC = 512                    # attention q-chunk
NQC = T // QC               # 4 q-chunks per batch
HT = T // 2                 # half-batch token span (one AllToAll each)
TOKH = HT // W              # 128 tokens per core per half-batch exchange
SCALE = 1.0 / float(np.sqrt(Dh))

F32 = mybir.dt.float32
BF16 = mybir.dt.bfloat16
FP8 = mybir.dt.float8e4
DR = mybir.MatmulPerfMode.DoubleRow
EXPB = -2.0                 # exp bias shift: keeps exp(s+EXPB) < fp8e4 max (240)
MULT = mybir.AluOpType.mult
ADD = mybir.AluOpType.add


def _build():
    nc = bacc.Bacc("TRN2", target_bir_lowering=False, debug=False,
                   enable_asserts=True, num_devices=W)
    xT = nc.dram_tensor("xT", [D, NT], BF16, kind="ExternalInput").ap()
    wqkv = nc.dram_tensor("wqkv", [D, CQKV], BF16, kind="ExternalInput").ap()
    bqkv = nc.dram_tensor("bqkv", [2 * HL * 128], F32, kind="ExternalInput").ap()
    wout = nc.dram_tensor("wout", [D, D], BF16, kind="ExternalInput").ap()
    maskneg = nc.dram_tensor("maskneg", [128, 128], BF16, kind="ExternalInput").ap()
    bvbc = nc.dram_tensor("bvbc", [128, 2 * HL * Dh], F32, kind="ExternalInput").ap()
    boutbc = nc.dram_tensor("boutbc", [128, D], F32, kind="ExternalInput").ap()
    # rows [(b*2+half)*TOKH ...): tokens [half*HT + c*TOKH ...) of batch b
    out = nc.dram_tensor("out", [B * 2 * TOKH, D], F32, kind="ExternalOutput").ap()

    xT_v = xT.rearrange("(ko p) t -> p ko t", p=128)
    wqkv_v = wqkv.rearrange("(ko p) c -> p ko c", p=128)
    wout_v = wout.rearrange("(ko p) c -> p ko c", p=128)

    with tile.TileContext(nc) as tc:
        with tc.tile_pool(name="persist", bufs=1) as persist, \
             tc.tile_pool(name="dram", bufs=1, space="DRAM") as dram_pool:
            mask_sb = persist.tile([128, 128], BF16, name="mask")   # 0 / -1e9
            ones8_sb = persist.tile([128, 2, 128], BF16, name="ones8")
            bqk_sb = persist.tile([128, 2 * HL], F32, name="bqk")
            expb_sb = persist.tile([128, 1], F32, name="expb")
            bv_sb = persist.tile([128, 2 * HL * Dh], F32, name="bv")  # (hl tb d)
            bout_sb = persist.tile([128, D], F32, name="bout")
            wqkv_sb = [persist.tile([128, CQKV], BF16, name=f"wqkv{ko}")
                       for ko in range(KO)]
            wout_sb = [persist.tile([128, D], BF16, name=f"wout{ko}")
                       for ko in range(KO)]
            qT = [persist.tile([128, HL, T], BF16, name=f"qT{b}") for b in range(B)]
            kT = [persist.tile([128, HL, T], BF16, name=f"kT{b}") for b in range(B)]
            v = [persist.tile([128, HL, T // 128, Dh], BF16, name=f"v{b}")
                 for b in range(B)]

            nc.gpsimd.memset(expb_sb[:], EXPB)
            nc.gpsimd.memset(ones8_sb[:], 1.0)

            a2a_in = [[dram_pool.tile([W, HL * 128, TOKH], BF16, name=f"a2a_in{b}{h}")
                       for h in range(2)] for b in range(B)]
            a2a_out = [[dram_pool.tile([W, HL * 128, TOKH], BF16, name=f"a2a_out{b}{h}")
                        for h in range(2)] for b in range(B)]

            with tc.tile_pool(name="x_pool", bufs=2) as x_pool, \
                 tc.tile_pool(name="ex_pool", bufs=3) as ex_pool, \
                 tc.tile_pool(name="rden_pool", bufs=2) as rden_pool, \
                 tc.tile_pool(name="attc_pool", bufs=3) as attc_pool, \
                 tc.tile_pool(name="attall_pool", bufs=4) as attall_pool, \
                 tc.tile_pool(name="o_pool", bufs=3) as o_pool, \
                 tc.tile_pool(name="proj_psum", bufs=2, space="PSUM") as proj_psum, \
                 tc.tile_pool(name="s_psum", bufs=2, space="PSUM") as s_psum, \
                 tc.tile_pool(name="av_psum", bufs=2, space="PSUM") as av_psum, \
                 tc.tile_pool(name="dout_psum", bufs=2, space="PSUM") as dout_psum:

                def prefetch_x(b, ci):
                    t0 = b * T + ci * TC
                    x_sb = x_pool.tile([128, KO, TC], BF16, name="x_sb")
                    nc.sync.dma_start(x_sb[:], xT_v[:, :, t0:t0 + TC])
                    return x_sb

                def emit_proj_chunk(qkv, b, ci, x_pre=None):
                    """Project one 512-token chunk of batch b into qT/kT/v."""
                    qTb, kTb, vb = qkv
                    x_sb = x_pre if x_pre is not None else prefetch_x(b, ci)
                    for ccp in range(2):            # 0: Q (hl0,hl1), 1: K
                        ps = proj_psum.tile([128, 2, TC], F32, name="proj_ps")
                        for i in range(2):
                            cc = ccp * 2 + i
                            for ko in range(KO):
                                nc.tensor.matmul(
                                    ps[:, i, :],
                                    wqkv_sb[ko][:, cc * 128:(cc + 1) * 128],
                                    x_sb[:, ko, :],
                                    start=(ko == 0), stop=(ko == KO - 1))
                        dest = qTb if ccp == 0 else kTb
                        for i in range(2):
                            nc.vector.tensor_scalar_add(
                                dest[:, i, ci * TC:(ci + 1) * TC], ps[:, i, :],
                                bqk_sb[:, ccp * 2 + i:ccp * 2 + i + 1])
                    ps = proj_psum.tile([128, 2, TC], F32, name="proj_ps")
                    for tb in range(TC // 128):
                        for ko in range(KO):
                            nc.tensor.matmul(
                                ps[:, tb, 0:256],
                                x_sb[:, ko, tb * 128:(tb + 1) * 128],
                                wqkv_sb[ko][:, 2 * HL * 128:],
                                start=(ko == 0), stop=(ko == KO - 1))
                    vidx = ci * (TC // 128)
                    nc.vector.tensor_tensor(
                        vb[:, :, vidx:vidx + 2, :],
                        ps[:, :, 0:256].rearrange("p tb (hl d) -> p hl tb d",
                                                  hl=HL),
                        bv_sb[:].rearrange("p (hl tb d) -> p hl tb d",
                                           hl=HL, tb=2),
                        ADD)

                def emit_attn_group(qkv, b, hl, qc):
                    qTb, kTb, vb = qkv
                    """One (head, q-chunk) group: S^T -> exp -> P^T V, denom via
                    ones-matmul; normalized att^T chunk DMAed to a2a_in.

                    Off-diagonal k-blocks are processed in pairs as fp8
                    DoubleRow matmuls (2x PE rate); diagonal blocks get an
                    additive -1e9 causal mask on the fp32 scores pre-exp."""
                    q0 = qc * QC
                    nkb = (qc + 1) * (QC // 128)
                    ndiag = QC // 128
                    npair = (nkb - ndiag) // 2
                    ps_av = av_psum.tile([128, QC], F32, name="ps_av")
                    ps_d = dout_psum.tile([128, QC], F32, name="ps_do")
                    units = [("pair", 2 * i) for i in range(npair)] \
                        + [("diag", 2 * npair + j) for j in range(ndiag)]
                    exs = {}

                    def emit_S_unit(u):
                        kind, kb = u
                        if kind == "pair":
                            ex2 = ex_pool.tile([128, 2, QC], BF16, name="ex2")
                            for t in range(2):
                                ps_s = s_psum.tile([128, QC], F32, name="ps_s")
                                nc.tensor.matmul(
                                    ps_s[:],
                                    kTb[:, hl, (kb + t) * 128:(kb + t + 1) * 128],
                                    qTb[:, hl, q0:q0 + QC],
                                    start=True, stop=True)
                                nc.scalar.activation(
                                    ex2[:, t, :], ps_s[:],
                                    mybir.ActivationFunctionType.Exp,
                                    scale=SCALE, bias=expb_sb[:])
                            exs[u] = ex2
                        else:
                            vs = (kb - qc * ndiag) * 128
                            ps_s = s_psum.tile([128, QC], F32, name="ps_s")
                            nc.tensor.matmul(
                                ps_s[:, vs:], kTb[:, hl, kb * 128:(kb + 1) * 128],
                                qTb[:, hl, q0 + vs:q0 + QC], start=True, stop=True)
                            nc.vector.tensor_tensor(
                                ps_s[:, vs:vs + 128], ps_s[:, vs:vs + 128],
                                mask_sb[:], ADD)
                            ex = ex_pool.tile([128, QC], BF16, name="ex")
                            nc.scalar.activation(
                                ex[:, vs:], ps_s[:, vs:],
                                mybir.ActivationFunctionType.Exp,
                                scale=SCALE, bias=expb_sb[:])
                            exs[u] = (ex, vs)

                    def emit_PV_unit(u, first, last):
                        kind, kb = u
                        if kind == "pair":
                            ex2 = exs[u]
                            nc.tensor.matmul(
                                ps_av[:], vb[:, hl, kb, :], ex2[:, 0, :],
                                start=first, stop=False)
                            nc.tensor.matmul(
                                ps_av[:], vb[:, hl, kb + 1, :], ex2[:, 1, :],
                                start=False, stop=last)
                            nc.tensor.matmul(
                                ps_d[:], ones8_sb[:, 0, :], ex2[:, 0, :],
                                start=first, stop=False)
                            nc.tensor.matmul(
                                ps_d[:], ones8_sb[:, 1, :], ex2[:, 1, :],
                                start=False, stop=last)
                        else:
                            ex, vs = exs[u]
                            nc.tensor.matmul(
                                ps_av[:, vs:], vb[:, hl, kb, :], ex[:, vs:],
                                start=first, stop=last)
                            nc.tensor.matmul(
                                ps_d[:, vs:], ones8_sb[:, 0, :], ex[:, vs:],
                                start=first, stop=last)

                    emit_S_unit(units[0])
                    for j in range(1, len(units)):
                        emit_S_unit(units[j])
                        emit_PV_unit(units[j - 1], j == 1, False)
                    emit_PV_unit(units[-1], len(units) == 1, True)

                    rden = rden_pool.tile([128, QC], F32, name="rden")
                    nc.vector.reciprocal_approx_fast(rden[:], ps_d[:])
                    attc = attc_pool.tile([128, QC], BF16, name="attc")
                    nc.vector.tensor_tensor(attc[:], ps_av[:], rden[:], MULT)
                    h = qc // 2
                    view = a2a_in[b][h].rearrange(
                        "(hh rr) (hl p) t -> p hl hh rr t",
                        hh=2, rr=W // 2, hl=HL, p=128)
                    nc.gpsimd.dma_start(
                        view[:, hl, qc % 2],
                        attc[:].rearrange("p (rr t) -> p rr t", rr=W // 2))

                def emit_a2a(b, h):
                    nc.gpsimd.collective_compute(
                        "AllToAll", mybir.AluOpType.bypass,
                        replica_groups=[list(range(W))],
                        ins=[a2a_in[b][h][:].opt()], outs=[a2a_out[b][h][:].opt()])

                def emit_attall(b, h, slot):
                    ga = attall_pool.tile([128, KO, TOKH], BF16, name="attall")
                    nc.sync.dma_start(
                        ga[:],
                        a2a_out[b][h].rearrange("r (hl p) t -> p (r hl) t",
                                                hl=HL, p=128))
                    slot[(b, h)] = ga

                def emit_outproj(b, h, slot):
                    ga = slot[(b, h)]
                    for colc in range(D // 512):
                        ps_o = dout_psum.tile([128, 512], F32, name="ps_do")
                        for ko in range(KO):
                            nc.tensor.matmul(
                                ps_o[:], ga[:, ko, :],
                                wout_sb[ko][:, colc * 512:(colc + 1) * 512],
                                start=(ko == 0), stop=(ko == KO - 1))
                        o_sb = o_pool.tile([128, 512], F32, name="o_sb")
                        nc.vector.tensor_tensor(
                            o_sb[:], ps_o[:],
                            bout_sb[:, colc * 512:(colc + 1) * 512], ADD)
                        nc.sync.dma_start(
                            out[(b * 2 + h) * TOKH:(b * 2 + h + 1) * TOKH,
                                colc * 512:(colc + 1) * 512],
                            o_sb[:])

                ga = {}
                # x chunk 0 DMA first (1MB, gates the first matmul), then the
                # wqkv tiles; wout (8MB) is held back until proj chunk 2's
                # output exists so it can't steal startup HBM bandwidth
                x0_sb = x_pool.tile([128, KO, TC], BF16, name="x_sb")
                nc.sync.dma_start(x0_sb[:, 0:KO // 2], xT_v[:, 0:KO // 2, 0:TC])
                nc.scalar.dma_start(x0_sb[:, KO // 2:], xT_v[:, KO // 2:, 0:TC])
                x_pre = {(0, 0): x0_sb}
                nc.sync.dma_start(mask_sb[:], maskneg)
                nc.sync.dma_start(bqk_sb[:], bqkv.rearrange("(cc p) -> p cc", p=128))
                nc.sync.dma_start(bv_sb[:], bvbc)
                for ko in range(KO):
                    nc.gpsimd.dma_start(wqkv_sb[ko][:], wqkv_v[:, ko, :])

                def emit_wout_load(qT0, ci):
                    # Tile hoists dependency-free DMAs to t=0, which would let
                    # this 8.5MB steal startup HBM bandwidth from the critical
                    # x/wqkv transfers. Gate each pair of tiles behind a write
                    # that depends on batch-0's qT chunk ci (WAW on the DMA),
                    # spreading the load across the whole batch-0 phase.
                    tq = ci * TC + 1
                    if ci == 0:
                        nc.vector.tensor_scalar_add(
                            bout_sb[:, 0:1], qT0[:, 0, tq:tq + 1], 0.0)
                        nc.scalar.dma_start(bout_sb[:], boutbc)
                    for ko in (2 * ci, 2 * ci + 1):
                        nc.vector.tensor_scalar_add(
                            wout_sb[ko][:, 0:1], qT0[:, 0, tq:tq + 1], 0.0)
                        nc.scalar.dma_start(wout_sb[ko][:], wout_v[:, ko, :])

                qkv0 = (qT[0], kT[0], v[0])
                qkv1 = (qT[1], kT[1], v[1])
                # batch 0: proj + attention interleaved, A2As fire mid-phase
                for ci in range(NTC_B):
                    emit_proj_chunk(qkv0, 0, ci, x_pre.get((0, ci)))
                    emit_wout_load(qT[0], ci)
                    if ci % 2 == 1:
                        qc = ci // 2
                        emit_attn_group(qkv0, 0, 0, qc)
                        emit_attn_group(qkv0, 0, 1, qc)
                        if qc == 1:
                            emit_a2a(0, 0)
                        if qc == 3:
                            emit_a2a(0, 1)
                # batch 1: same; out-projections all run in the tail
                for ci in range(NTC_B):
                    emit_proj_chunk(qkv1, 1, ci)
                    if ci == 2:
                        emit_attall(0, 0, ga)
                    if ci == 4:
                        emit_attall(0, 1, ga)
                    if ci % 2 == 1:
                        qc = ci // 2
                        emit_attn_group(qkv1, 1, 0, qc)
                        emit_attn_group(qkv1, 1, 1, qc)
                        if qc == 1:
                            emit_a2a(1, 0)
                        if qc == 3:
                            emit_a2a(1, 1)
                # tail: batch-0's out-projections are independent of batch-1's
                # A2As, so ~35us of PE work hides the last A2A's latency
                # before outproj(1,*) needs its data
                emit_attall(1, 0, ga)
                emit_attall(1, 1, ga)
                emit_outproj(0, 0, ga)
                emit_outproj(0, 1, ga)
                emit_outproj(1, 0, ga)
                emit_outproj(1, 1, ga)
    nc.compile()
    return nc


_CACHED_NC = None


def kernel(x, Wqkv, bqkv, Wout, bout):
    global _CACHED_NC
    x = np.asarray(x, dtype=np.float32)
    Wqkv = np.asarray(Wqkv, dtype=np.float32)
    bqkv = np.asarray(bqkv, dtype=np.float32)
    Wout = np.asarray(Wout, dtype=np.float32)
    bout = np.asarray(bout, dtype=np.float32)

    if _CACHED_NC is None:
        _CACHED_NC = _build()
    nc = _CACHED_NC

    bf16 = ml_dtypes.bfloat16
    xT = np.ascontiguousarray(x.reshape(NT, D).T).astype(bf16)   # [D, NT]
    wq4 = Wqkv.reshape(D, 3, H, Dh)                 # col = (which, head, dh)
    bq4 = bqkv.reshape(3, H, Dh)
    kl = np.arange(128)[:, None]
    jl = np.arange(128)[None, :]
    maskneg = np.where(jl >= kl, 0.0, -1e9).astype(bf16)
    fp8 = ml_dtypes.float8_e4m3fn
    wout_bf = Wout.astype(bf16)
    boutbc = np.tile(bout[None, :], (128, 1)).astype(np.float32)

    in_maps = []
    for c in range(W):
        wshard = np.ascontiguousarray(
            wq4[:, :, HL * c:HL * c + HL, :].reshape(D, CQKV)).astype(bf16)
        bshard_qk = np.ascontiguousarray(
            bq4[0:2, HL * c:HL * c + HL, :].reshape(2 * HL * 128)
        ).astype(np.float32)
        bshard_v = bq4[2, HL * c:HL * c + HL, :]                  # [HL, Dh]
        bvbc = np.ascontiguousarray(np.broadcast_to(
            bshard_v.reshape(1, HL, 1, Dh), (128, HL, 2, Dh)
        ).reshape(128, 2 * HL * Dh)).astype(np.float32)
        in_maps.append({
            "xT": xT, "wqkv": wshard, "bqkv": bshard_qk,
            "wout": wout_bf, "maskneg": maskneg,
            "bvbc": bvbc,
            "boutbc": boutbc,
        })

    res = run_bass_kernel_spmd(nc, in_maps, core_ids=list(range(W)))
    # res[c]["out"] rows [(b*2+h)*TOKH ...) = tokens [h*HT + c*TOKH ...) of batch b
    full = np.empty((B, T, D), np.float32)
    for c in range(W):
        for b in range(B):
            for h in range(2):
                full[b, h * HT + c * TOKH:h * HT + (c + 1) * TOKH] = \
                    res.results[c]["out"][(b * 2 + h) * TOKH:(b * 2 + h + 1) * TOKH]
    return full



# revision 10
# speedup vs baseline: 1.1704x; 1.1704x over previous
"""Causal self-attention kernel for 8 Trainium2 NeuronCores.

Problem: B=2, T=2048, D=2048, H=16, Dh=128, fp32 in/out.
  qkv = x @ Wqkv + bqkv ; per-head causal attention ; out = att @ Wout + bout

Sharding (tensor parallel over heads + AllToAll before out_proj):
  Core c owns heads {2c, 2c+1}. Each core computes Q^T/K^T (head-dim on
  partitions) and V (token-dim on partitions) for all 4096 tokens via the
  QKV projection with its column shard of Wqkv, runs causal attention
  locally (scores computed transposed: S^T[k,q], softmax reduction over k
  via a ones-matmul which also broadcasts the denominator), and produces
  att^T per batch. Four AllToAlls (one per half-batch of tokens)
  redistribute head-sharded -> token-sharded; core c projects its 128-token
  slices with the full Wout (resident in SBUF).

v3 precision/structure:
  - Hybrid precision driven by causal-window size: early tokens (chunk 0,
    t<512) have tiny attention windows where fp8 errors don't average out,
    so chunk 0 runs a full bf16 path (bf16 x/Wqkv/q/k/v, bf16 P for qc=0).
    Later chunks run fp8e4m3 with DoubleRow (2x PE rate): errors there
    redistribute/average over >=512-key softmax windows.
  - Proj chunks 1-3: Q/K and V projections are fp8 DoubleRow over 256-row
    d-pairs; q/k stored fp8, v stored fp8. Chunk 0: bf16 matmuls; k/v also
    stored as fp8 copies for use by later (large-window) q-chunks.
  - Attention qc>=1: uniform k-block PAIR units: S^T for both blocks ->
    one batched exp over [128,2,512] PSUM -> one DoubleRow fp8 PV matmul
    and one DoubleRow ones-matmul (denominator). Diagonal blocks get
    full-width additive causal masks (masked entries exp to 0) so they
    pair exactly like off-diagonal blocks. qc=0: same structure in bf16.
  - Chunk order per batch is 1,2,3,0 so the fp8 chunks (small weight
    footprint) start immediately while the 3MB bf16 weights stream in.
  - exp bias EXPB keeps exp(s+EXPB) << 240 (TRN fp8e4 max; Inf at 256).
"""

import numpy as np
import ml_dtypes

import concourse.bass as bass
import concourse.mybir as mybir
import concourse.tile as tile
from concourse import bacc
from concourse.bass_utils import run_bass_kernel_spmd

B, T, D, H, Dh = 2, 2048, 2048, 16, 128
NT = B * T                  # 4096 tokens total
W = 8                       # cores
HL = H // W                 # 2 heads per core
CQK = 4 * 128               # q/k columns per core (2 heads x (q,k))
CV = 2 * Dh                 # v columns per core
KO = D // 128               # 16 contraction subtiles (128 rows each)
KP = D // 256               # 8 contraction pair-tiles (DoubleRow)
TC = 512                    # proj token chunk == attention q-chunk
NTC_B = T // TC             # 4 chunks per batch
QC = 512
NKB = T // 128              # 16 k-blocks per batch
HT = T // 2                 # half-batch token span (one AllToAll each)
TOKH = HT // W              # 128 tokens per core per half-batch exchange
SCALE = 1.0 / float(np.sqrt(Dh))

F32 = mybir.dt.float32
BF16 = mybir.dt.bfloat16
FP8 = mybir.dt.float8e4
DR = mybir.MatmulPerfMode.DoubleRow
EXPB = -2.5                 # exp bias shift: keeps exp(s+EXPB) << fp8e4 max
MULT = mybir.AluOpType.mult
ADD = mybir.AluOpType.add


def _build():
    nc = bacc.Bacc("TRN2", target_bir_lowering=False, debug=False,
                   enable_asserts=True, num_devices=W)
    x8d = nc.dram_tensor("x8d", [D, NT], FP8, kind="ExternalInput").ap()
    xbd = nc.dram_tensor("xbd", [D, B * TC], BF16, kind="ExternalInput").ap()
    w8qk = nc.dram_tensor("w8qk", [128, KP * 2 * CQK], FP8,
                          kind="ExternalInput").ap()
    w8vd = nc.dram_tensor("w8vd", [128, KP * 2 * CV], FP8,
                          kind="ExternalInput").ap()
    wqkvb = nc.dram_tensor("wqkvb", [D, CQK + CV], BF16,
                           kind="ExternalInput").ap()
    bqkd = nc.dram_tensor("bqkd", [128, 4], F32, kind="ExternalInput").ap()
    wout = nc.dram_tensor("wout", [D, D], BF16, kind="ExternalInput").ap()
    masks4 = nc.dram_tensor("masks4", [128, 4 * 512], BF16,
                            kind="ExternalInput").ap()
    ones2d = nc.dram_tensor("ones2d", [128, 256], FP8,
                            kind="ExternalInput").ap()
    onesbd = nc.dram_tensor("onesbd", [128, 128], BF16,
                            kind="ExternalInput").ap()
    bvbc = nc.dram_tensor("bvbc", [128, 2 * HL * Dh], F32,
                          kind="ExternalInput").ap()
    boutbc = nc.dram_tensor("boutbc", [128, D], BF16, kind="ExternalInput").ap()
    # rows [(b*2+half)*TOKH ...): tokens [half*HT + c*TOKH ...) of batch b
    out = nc.dram_tensor("out", [B * 2 * TOKH, D], F32, kind="ExternalOutput").ap()

    x8_v = x8d.rearrange("(kp two p) t -> p kp two t", kp=KP, two=2, p=128)
    xb_v = xbd.rearrange("(ko p) t -> p ko t", p=128)
    wqkvb_v = wqkvb.rearrange("(ko p) c -> p ko c", p=128)
    wout_v = wout.rearrange("(ko p) c -> p ko c", p=128)

    with tile.TileContext(nc) as tc:
        with tc.tile_pool(name="persist", bufs=1) as persist, \
             tc.tile_pool(name="dram", bufs=1, space="DRAM") as dram_pool:
            masks_sb = persist.tile([128, 4, 512], BF16, name="masks")
            ones2_sb = persist.tile([128, 2, 128], FP8, name="ones2")
            onesb_sb = persist.tile([128, 128], BF16, name="onesb")
            bqk_sb = persist.tile([128, 4], F32, name="bqk")
            expb_sb = persist.tile([128, 1], F32, name="expb")
            bv_sb = persist.tile([128, 2 * HL * Dh], F32, name="bv")
            bout_sb = persist.tile([128, D], BF16, name="bout")
            w8qk_sb = persist.tile([128, KP, 2, CQK], FP8, name="w8qk")
            w8v_sb = persist.tile([128, KP, 2, CV], FP8, name="w8v")
            wqkvb_sb = [persist.tile([128, CQK + CV], BF16, name=f"wqkvb{ko}")
                        for ko in range(KO)]
            wout_sb = [persist.tile([128, D], BF16, name=f"wout{ko}")
                       for ko in range(KO)]
            # fp8 q/k for qc>=1 (large windows); bf16 q/k for chunk 0
            qT8 = [persist.tile([128, HL, T], FP8, name=f"qT8{b}") for b in range(B)]
            kT8 = [persist.tile([128, HL, T], FP8, name=f"kT8{b}") for b in range(B)]
            qTe = [persist.tile([128, HL, TC], BF16, name=f"qTe{b}") for b in range(B)]
            kTe = [persist.tile([128, HL, TC], BF16, name=f"kTe{b}") for b in range(B)]
            vv = [persist.tile([128, HL, NKB, Dh], FP8, name=f"v{b}")
                  for b in range(B)]
            vve = [persist.tile([128, HL, 4, Dh], BF16, name=f"ve{b}")
                   for b in range(B)]
            xb0 = persist.tile([128, KO, TC], BF16, name="xb0")

            nc.gpsimd.memset(expb_sb[:], EXPB)

            a2a_in = [[dram_pool.tile([W, HL * 128, TOKH], BF16, name=f"a2a_in{b}{h}")
                       for h in range(2)] for b in range(B)]
            a2a_out = [[dram_pool.tile([W, HL * 128, TOKH], BF16, name=f"a2a_out{b}{h}")
                        for h in range(2)] for b in range(B)]

            with tc.tile_pool(name="x8_pool", bufs=2) as x8_pool, \
                 tc.tile_pool(name="ex_pool", bufs=3) as ex_pool, \
                 tc.tile_pool(name="rden_pool", bufs=1) as rden_pool, \
                 tc.tile_pool(name="attc_pool", bufs=3) as attc_pool, \
                 tc.tile_pool(name="attall_pool", bufs=3) as attall_pool, \
                 tc.tile_pool(name="o_pool", bufs=2) as o_pool, \
                 tc.tile_pool(name="qk_psum", bufs=2, space="PSUM") as qk_psum, \
                 tc.tile_pool(name="s_psum", bufs=2, space="PSUM") as s_psum, \
                 tc.tile_pool(name="av_psum", bufs=1, space="PSUM") as av_psum, \
                 tc.tile_pool(name="den_psum", bufs=1, space="PSUM") as den_psum:

                def emit_proj_chunk8(b, ci, split=False):
                    """fp8 DoubleRow projection of one 512-token chunk (ci>=1)."""
                    t0 = b * T + ci * TC
                    x8_sb = x8_pool.tile([128, KP, 2, TC], FP8, name="x8_sb")
                    if split:
                        nc.sync.dma_start(x8_sb[:, 0:KP // 2],
                                          x8_v[:, 0:KP // 2, :, t0:t0 + TC])
                        nc.scalar.dma_start(x8_sb[:, KP // 2:],
                                            x8_v[:, KP // 2:, :, t0:t0 + TC])
                    else:
                        nc.sync.dma_start(x8_sb[:], x8_v[:, :, :, t0:t0 + TC])
                    for cc in range(4):             # 0,1: Q h0,h1; 2,3: K h0,h1
                        ps = qk_psum.tile([128, TC], F32, name="qk_ps", tag="qk")
                        for kp in range(KP):
                            nc.tensor.matmul(
                                ps[:],
                                w8qk_sb[:, kp, :, cc * 128:(cc + 1) * 128],
                                x8_sb[:, kp, :, :],
                                start=(kp == 0), stop=(kp == KP - 1),
                                perf_mode=DR)
                        dest = qT8[b] if cc < 2 else kT8[b]
                        nc.vector.tensor_scalar_add(
                            dest[:, cc % 2, ci * TC:(ci + 1) * TC], ps[:],
                            bqk_sb[:, cc:cc + 1])
                    for th in range(2):
                        psv = qk_psum.tile([128, 2, 256], F32, name="v_ps",
                                           tag="qk")
                        for tb in range(2):
                            tok = (th * 2 + tb) * 128
                            for kp in range(KP):
                                nc.tensor.matmul(
                                    psv[:, tb, :],
                                    x8_sb[:, kp, :, tok:tok + 128],
                                    w8v_sb[:, kp, :, :],
                                    start=(kp == 0), stop=(kp == KP - 1),
                                    perf_mode=DR)
                        vidx = ci * 4 + th * 2
                        nc.vector.tensor_tensor(
                            vv[b][:, :, vidx:vidx + 2, :],
                            psv[:].rearrange("p tb (hl d) -> p hl tb d", hl=HL),
                            bv_sb[:].rearrange("p (hl tb d) -> p hl tb d",
                                               hl=HL, tb=2),
                            ADD)

                def emit_proj_chunk_bf(b):
                    """bf16 projection of chunk 0 (small-window tokens).

                    Writes bf16 qTe/kTe/vve for the qc=0 attention group and
                    fp8 copies of k/v for later q-chunks' big-window reads."""
                    t0 = b * TC
                    nc.sync.dma_start(xb0[:, 0:KO // 2],
                                      xb_v[:, 0:KO // 2, t0:t0 + TC])
                    nc.scalar.dma_start(xb0[:, KO // 2:],
                                        xb_v[:, KO // 2:, t0:t0 + TC])
                    for cc in range(4):
                        ps = qk_psum.tile([128, TC], F32, name="qk_ps", tag="qk")
                        for ko in range(KO):
                            nc.tensor.matmul(
                                ps[:],
                                wqkvb_sb[ko][:, cc * 128:(cc + 1) * 128],
                                xb0[:, ko, :],
                                start=(ko == 0), stop=(ko == KO - 1))
                        dest = qTe[b] if cc < 2 else kTe[b]
                        nc.vector.tensor_scalar_add(
                            dest[:, cc % 2, :], ps[:], bqk_sb[:, cc:cc + 1])
                        if cc >= 2:     # fp8 copy of early k for qc>=1 groups
                            nc.vector.tensor_scalar_add(
                                kT8[b][:, cc % 2, 0:TC], ps[:],
                                bqk_sb[:, cc:cc + 1])
                    for th in range(2):
                        psv = qk_psum.tile([128, 2, 256], F32, name="v_ps",
                                           tag="qk")
                        for tb in range(2):
                            tok = (th * 2 + tb) * 128
                            for ko in range(KO):
                                nc.tensor.matmul(
                                    psv[:, tb, :],
                                    xb0[:, ko, tok:tok + 128],
                                    wqkvb_sb[ko][:, CQK:],
                                    start=(ko == 0), stop=(ko == KO - 1))
                        vidx = th * 2
                        bvr = bv_sb[:].rearrange("p (hl tb d) -> p hl tb d",
                                                 hl=HL, tb=2)
                        psr = psv[:].rearrange("p tb (hl d) -> p hl tb d", hl=HL)
                        nc.vector.tensor_tensor(
                            vve[b][:, :, vidx:vidx + 2, :], psr, bvr, ADD)
                        nc.vector.tensor_tensor(
                            vv[b][:, :, vidx:vidx + 2, :], psr, bvr, ADD)

                def emit_attn_group(b, hl, qc):
                    """One (head, q-chunk) group, all k-blocks as pair units.

                    qc>=1: S^T pair (fp8 q/k) -> batched exp -> fp8 ex2 ->
                    one DoubleRow PV + one DoubleRow ones-matmul (denom).
                    qc=0: same shape in bf16 (qTe/kTe/vve, bf16 ex2, plain
                    matmuls) because small windows don't average fp8 noise.
                    Diagonal blocks get full-width additive causal masks
                    before the exp, so masked entries exp to 0."""
                    bf = (qc == 0)
                    q0 = qc * QC
                    npu = (qc + 1) * 2          # pair units (256 keys each)
                    ps_av = av_psum.tile([128, QC], F32, name="ps_av")
                    ps_d = den_psum.tile([128, QC], F32, name="ps_d")
                    exs = {}

                    def emit_S(u):
                        kb = 2 * u
                        ps2 = s_psum.tile([128, 2, QC], F32, name="ps_s2")
                        for t in range(2):
                            if bf:
                                nc.tensor.matmul(
                                    ps2[:, t, :],
                                    kTe[b][:, hl, (kb + t) * 128:(kb + t + 1) * 128],
                                    qTe[b][:, hl, :], start=True, stop=True)
                            else:
                                nc.tensor.matmul(
                                    ps2[:, t, :],
                                    kT8[b][:, hl, (kb + t) * 128:(kb + t + 1) * 128],
                                    qT8[b][:, hl, q0:q0 + QC],
                                    start=True, stop=True)
                            dv = kb + t - qc * 4
                            if dv >= 0:
                                nc.vector.tensor_tensor(
                                    ps2[:, t, :], ps2[:, t, :],
                                    masks_sb[:, dv, :], ADD)
                        ex2 = ex_pool.tile([128, 2, QC], BF16 if bf else FP8,
                                           name="ex2", tag="ex2")
                        nc.scalar.activation(
                            ex2[:], ps2[:],
                            mybir.ActivationFunctionType.Exp,
                            scale=SCALE, bias=expb_sb[:])
                        exs[u] = ex2

                    def emit_PV(u, first, last):
                        kb = 2 * u
                        ex2 = exs.pop(u)
                        if bf:
                            for t in range(2):
                                nc.tensor.matmul(
                                    ps_av[:], vve[b][:, hl, kb + t, :],
                                    ex2[:, t, :],
                                    start=first and t == 0, stop=last and t == 1)
                                nc.tensor.matmul(
                                    ps_d[:], onesb_sb[:], ex2[:, t, :],
                                    start=first and t == 0, stop=last and t == 1)
                        else:
                            nc.tensor.matmul(
                                ps_av[:], vv[b][:, hl, kb:kb + 2, :], ex2[:],
                                start=first, stop=last, perf_mode=DR)
                            nc.tensor.matmul(
                                ps_d[:], ones2_sb[:], ex2[:],
                                start=first, stop=last, perf_mode=DR)

                    emit_S(0)
                    for j in range(1, npu):
                        emit_S(j)
                        emit_PV(j - 1, j == 1, False)
                    emit_PV(npu - 1, npu == 1, True)

                    rden = rden_pool.tile([128, QC], F32, name="rden")
                    nc.vector.reciprocal_approx_fast(rden[:], ps_d[:])
                    attc = attc_pool.tile([128, QC], BF16, name="attc")
                    nc.vector.tensor_tensor(attc[:], ps_av[:], rden[:], MULT)
                    h = qc // 2
                    view = a2a_in[b][h].rearrange(
                        "(hh rr) (hl p) t -> p hl hh rr t",
                        hh=2, rr=W // 2, hl=HL, p=128)
                    nc.gpsimd.dma_start(
                        view[:, hl, qc % 2],
                        attc[:].rearrange("p (rr t) -> p rr t", rr=W // 2))

                def emit_a2a(b, h):
                    nc.gpsimd.collective_compute(
                        "AllToAll", mybir.AluOpType.bypass,
                        replica_groups=[list(range(W))],
                        ins=[a2a_in[b][h][:].opt()], outs=[a2a_out[b][h][:].opt()])

                def emit_attall(b, h, slot):
                    ga = attall_pool.tile([128, KO, TOKH], BF16, name="attall")
                    nc.scalar.dma_start(
                        ga[:],
                        a2a_out[b][h].rearrange("r (hl p) t -> p (r hl) t",
                                                hl=HL, p=128))
                    slot[(b, h)] = ga

                def emit_outproj(b, h, slot):
                    ga = slot[(b, h)]
                    for colc in range(D // 512):
                        ps_o = qk_psum.tile([128, 512], F32, name="qk_ps",
                                            tag="qk")
                        for ko in range(KO):
                            nc.tensor.matmul(
                                ps_o[:], ga[:, ko, :],
                                wout_sb[ko][:, colc * 512:(colc + 1) * 512],
                                start=(ko == 0), stop=(ko == KO - 1))
                        o_sb = o_pool.tile([128, 512], F32, name="o_sb")
                        nc.vector.tensor_tensor(
                            o_sb[:], ps_o[:],
                            bout_sb[:, colc * 512:(colc + 1) * 512], ADD)
                        nc.scalar.dma_start(
                            out[(b * 2 + h) * TOKH:(b * 2 + h + 1) * TOKH,
                                colc * 512:(colc + 1) * 512],
                            o_sb[:])

                ga = {}
                # startup: fp8 weights first on scalar (the first chunk's
                # matmuls need them), x8 chunk on sync; the 3MB bf16
                # chunk-0 weights stream on gpsimd (not needed until ~60us
                # in); wout (8.5MB) is gated behind proj progress below.
                nc.scalar.dma_start(
                    w8qk_sb[:],
                    w8qk.rearrange("p (kp two c) -> p kp two c", kp=KP, two=2))
                nc.scalar.dma_start(
                    w8v_sb[:],
                    w8vd.rearrange("p (kp two c) -> p kp two c", kp=KP, two=2))
                emit_proj_chunk8(0, 1)
                nc.sync.dma_start(bqk_sb[:], bqkd)
                nc.sync.dma_start(
                    masks_sb[:], masks4.rearrange("p (dv q) -> p dv q", dv=4))
                nc.sync.dma_start(
                    ones2_sb[:], ones2d.rearrange("p (two q) -> p two q", two=2))
                nc.sync.dma_start(onesb_sb[:], onesbd)
                nc.sync.dma_start(bv_sb[:], bvbc)
                for ko in range(KO):
                    nc.gpsimd.dma_start(wqkvb_sb[ko][:], wqkvb_v[:, ko, :])

                def emit_wout_load(step):
                    # Gate each wout tile behind proj progress (WAW on the
                    # DMA) so the 8.5MB can't steal startup HBM bandwidth.
                    gate = (qTe[0][:, 0, 1:2] if step == 3
                            else qT8[0][:, 0, (step + 1) * TC + 1:(step + 1) * TC + 2])
                    if step == 0:
                        nc.vector.tensor_scalar_add(bout_sb[:, 0:1], gate, 0.0)
                        nc.sync.dma_start(bout_sb[:], boutbc)
                    for ko in range(4 * step, 4 * step + 4):
                        nc.vector.tensor_scalar_add(
                            wout_sb[ko][:, 0:1], gate, 0.0)
                        nc.sync.dma_start(wout_sb[ko][:], wout_v[:, ko, :])

                emit_wout_load(0)
                emit_proj_chunk8(0, 2)
                emit_wout_load(1)
                emit_proj_chunk8(0, 3)
                emit_wout_load(2)
                emit_proj_chunk_bf(0)
                emit_wout_load(3)
                # batch-0 attention interleaved with batch-1 projection
                emit_attn_group(0, 0, 0)
                emit_attn_group(0, 1, 0)
                emit_proj_chunk8(1, 1)
                emit_attn_group(0, 0, 1)
                emit_attn_group(0, 1, 1)
                emit_a2a(0, 0)
                emit_proj_chunk8(1, 2)
                emit_attn_group(0, 0, 2)
                emit_attn_group(0, 1, 2)
                emit_proj_chunk8(1, 3)
                emit_attn_group(0, 0, 3)
                emit_attn_group(0, 1, 3)
                emit_a2a(0, 1)
                emit_proj_chunk_bf(1)
                # batch-1 attention; batch-0 out-projections fill scalar gaps
                emit_attn_group(1, 0, 0)
                emit_attn_group(1, 1, 0)
                emit_attall(0, 0, ga)
                emit_attn_group(1, 0, 1)
                emit_attn_group(1, 1, 1)
                emit_a2a(1, 0)
                emit_attall(0, 1, ga)
                emit_attn_group(1, 0, 2)
                emit_attn_group(1, 1, 2)
                emit_outproj(0, 0, ga)
                emit_attn_group(1, 0, 3)
                emit_attn_group(1, 1, 3)
                emit_a2a(1, 1)
                emit_outproj(0, 1, ga)
                # tail: only batch-1's out-projections remain after its A2As
                emit_attall(1, 0, ga)
                emit_outproj(1, 0, ga)
                emit_attall(1, 1, ga)
                emit_outproj(1, 1, ga)
    nc.compile()
    return nc


_CACHED_NC = None


def kernel(x, Wqkv, bqkv, Wout, bout):
    global _CACHED_NC
    x = np.asarray(x, dtype=np.float32)
    Wqkv = np.asarray(Wqkv, dtype=np.float32)
    bqkv = np.asarray(bqkv, dtype=np.float32)
    Wout = np.asarray(Wout, dtype=np.float32)
    bout = np.asarray(bout, dtype=np.float32)

    if _CACHED_NC is None:
        _CACHED_NC = _build()
    nc = _CACHED_NC

    bf16 = ml_dtypes.bfloat16
    fp8 = ml_dtypes.float8_e4m3fn
    xTf = np.ascontiguousarray(x.reshape(NT, D).T)              # [D, NT] f32
    x8 = xTf.astype(fp8)
    xb = np.ascontiguousarray(
        np.concatenate([xTf[:, b * T:b * T + TC] for b in range(B)], axis=1)
    ).astype(bf16)                                              # [D, B*TC]
    wq4 = Wqkv.reshape(D, 3, H, Dh)                 # col = (which, head, dh)
    bq4 = bqkv.reshape(3, H, Dh)
    kl = np.arange(128)[:, None]
    ql = np.arange(512)[None, :]
    masks4 = np.stack(
        [np.where(ql >= dv * 128 + kl, 0.0, -1e9) for dv in range(4)],
        axis=1).reshape(128, 4 * 512).astype(bf16)
    ones2 = np.ones((128, 256), dtype=fp8)
    onesb = np.ones((128, 128), dtype=bf16)
    wout_bf = Wout.astype(bf16)
    boutbc = np.tile(bout[None, :], (128, 1)).astype(bf16)

    def dr_pack(w):
        # [D, C] -> [128, (kp two c)] matching the DoubleRow stationary AP
        cdim = w.shape[1]
        return np.ascontiguousarray(
            w.reshape(KP, 2, 128, cdim).transpose(2, 0, 1, 3)
            .reshape(128, KP * 2 * cdim))

    in_maps = []
    for c in range(W):
        wqk = np.ascontiguousarray(
            wq4[:, 0:2, HL * c:HL * c + HL, :].reshape(D, CQK))
        wv = np.ascontiguousarray(
            wq4[:, 2, HL * c:HL * c + HL, :].reshape(D, CV))
        wfull = np.ascontiguousarray(
            np.concatenate([wqk, wv], axis=1)).astype(bf16)     # [D, 768]
        bshard_qk = np.ascontiguousarray(
            bq4[0:2, HL * c:HL * c + HL, :].reshape(CQK)).astype(np.float32)
        bqk_pc = np.ascontiguousarray(
            bshard_qk.reshape(4, 128).T).astype(np.float32)     # [128, 4]
        bshard_v = bq4[2, HL * c:HL * c + HL, :]                  # [HL, Dh]
        bvbc = np.ascontiguousarray(np.broadcast_to(
            bshard_v.reshape(1, HL, 1, Dh), (128, HL, 2, Dh)
        ).reshape(128, 2 * HL * Dh)).astype(np.float32)
        in_maps.append({
            "x8d": x8, "xbd": xb,
            "w8qk": dr_pack(wqk).astype(fp8),
            "w8vd": dr_pack(wv).astype(fp8),
            "wqkvb": wfull, "bqkd": bqk_pc, "wout": wout_bf,
            "masks4": masks4, "ones2d": ones2, "onesbd": onesb,
            "bvbc": bvbc, "boutbc": boutbc,
        })

    res = run_bass_kernel_spmd(nc, in_maps, core_ids=list(range(W)))
    # res[c]["out"] rows [(b*2+h)*TOKH ...) = tokens [h*HT + c*TOKH ...) of batch b
    full = np.empty((B, T, D), np.float32)
    for c in range(W):
        for b in range(B):
            for h in range(2):
                full[b, h * HT + c * TOKH:h * HT + (c + 1) * TOKH] = \
                    res.results[c]["out"][(b * 2 + h) * TOKH:(b * 2 + h + 1) * TOKH]
    return full


# revision 11
# speedup vs baseline: 1.2429x; 1.0619x over previous
"""Causal self-attention kernel for 8 Trainium2 NeuronCores.

Problem: B=2, T=2048, D=2048, H=16, Dh=128, fp32 in/out.
  qkv = x @ Wqkv + bqkv ; per-head causal attention ; out = att @ Wout + bout

Sharding (tensor parallel over heads + AllToAll before out_proj):
  Core c owns heads {2c, 2c+1}. Each core computes Q^T/K^T (head-dim on
  partitions) and V (token-dim on partitions) for all 4096 tokens via the
  QKV projection with its column shard of Wqkv, runs causal attention
  locally (scores computed transposed: S^T[k,q], softmax reduction over k
  via a ones-matmul which also broadcasts the denominator), and produces
  att^T per batch. Four AllToAlls (one per half-batch of tokens)
  redistribute head-sharded -> token-sharded; core c projects its 128-token
  slices with the full Wout (resident in SBUF).

v3 precision/structure:
  - Hybrid precision driven by causal-window size: early tokens (chunk 0,
    t<512) have tiny attention windows where fp8 errors don't average out,
    so chunk 0 runs a full bf16 path (bf16 x/Wqkv/q/k/v, bf16 P for qc=0).
    Later chunks run fp8e4m3 with DoubleRow (2x PE rate): errors there
    redistribute/average over >=512-key softmax windows.
  - Proj chunks 1-3: Q/K and V projections are fp8 DoubleRow over 256-row
    d-pairs; q/k stored fp8, v stored fp8. Chunk 0: bf16 matmuls; k/v also
    stored as fp8 copies for use by later (large-window) q-chunks.
  - Attention qc>=1: uniform k-block PAIR units: S^T for both blocks ->
    one batched exp over [128,2,512] PSUM -> one DoubleRow fp8 PV matmul
    and one DoubleRow ones-matmul (denominator). Diagonal blocks get
    full-width additive causal masks (masked entries exp to 0) so they
    pair exactly like off-diagonal blocks. qc=0: same structure in bf16.
  - Chunk order per batch is 1,2,3,0 so the fp8 chunks (small weight
    footprint) start immediately while the 3MB bf16 weights stream in.
  - exp bias EXPB keeps exp(s+EXPB) << 240 (TRN fp8e4 max; Inf at 256).
"""

import numpy as np
import ml_dtypes

import concourse.bass as bass
import concourse.mybir as mybir
import concourse.tile as tile
from concourse import bacc
from concourse.bass_utils import run_bass_kernel_spmd

B, T, D, H, Dh = 2, 2048, 2048, 16, 128
NT = B * T                  # 4096 tokens total
W = 8                       # cores
HL = H // W                 # 2 heads per core
CQK = 4 * 128               # q/k columns per core (2 heads x (q,k))
CV = 2 * Dh                 # v columns per core
KO = D // 128               # 16 contraction subtiles (128 rows each)
KP = D // 256               # 8 contraction pair-tiles (DoubleRow)
TC = 512                    # proj token chunk == attention q-chunk
NTC_B = T // TC             # 4 chunks per batch
QC = 512
NKB = T // 128              # 16 k-blocks per batch
HT = T // 2                 # half-batch token span (one AllToAll each)
TOKH = HT // W              # 128 tokens per core per half-batch exchange
SCALE = 1.0 / float(np.sqrt(Dh))

F32 = mybir.dt.float32
BF16 = mybir.dt.bfloat16
FP8 = mybir.dt.float8e4
DR = mybir.MatmulPerfMode.DoubleRow
EXPB = -2.5                 # exp bias shift: keeps exp(s+EXPB) << fp8e4 max
MULT = mybir.AluOpType.mult
ADD = mybir.AluOpType.add


def _build():
    nc = bacc.Bacc("TRN2", target_bir_lowering=False, debug=False,
                   enable_asserts=True, num_devices=W)
    x8d = nc.dram_tensor("x8d", [D, NT], FP8, kind="ExternalInput").ap()
    xbd = nc.dram_tensor("xbd", [D, B * TC], BF16, kind="ExternalInput").ap()
    w8qk = nc.dram_tensor("w8qk", [128, KP * 2 * CQK], FP8,
                          kind="ExternalInput").ap()
    w8vd = nc.dram_tensor("w8vd", [128, KP * 2 * CV], FP8,
                          kind="ExternalInput").ap()
    wqkvb = nc.dram_tensor("wqkvb", [D, CQK + CV], BF16,
                           kind="ExternalInput").ap()
    bqkd = nc.dram_tensor("bqkd", [128, 4], F32, kind="ExternalInput").ap()
    wout = nc.dram_tensor("wout", [D, D], BF16, kind="ExternalInput").ap()
    masks4 = nc.dram_tensor("masks4", [128, 4 * 512], BF16,
                            kind="ExternalInput").ap()
    ones2d = nc.dram_tensor("ones2d", [128, 256], FP8,
                            kind="ExternalInput").ap()
    onesbd = nc.dram_tensor("onesbd", [128, 128], BF16,
                            kind="ExternalInput").ap()
    bvbc = nc.dram_tensor("bvbc", [128, 2 * HL * Dh], F32,
                          kind="ExternalInput").ap()
    boutbc = nc.dram_tensor("boutbc", [128, D], BF16, kind="ExternalInput").ap()
    # rows [(b*2+half)*TOKH ...): tokens [half*HT + c*TOKH ...) of batch b
    out = nc.dram_tensor("out", [B * 2 * TOKH, D], F32, kind="ExternalOutput").ap()

    x8_v = x8d.rearrange("(kp two p) t -> p kp two t", kp=KP, two=2, p=128)
    xb_v = xbd.rearrange("(ko p) t -> p ko t", p=128)
    wqkvb_v = wqkvb.rearrange("(ko p) c -> p ko c", p=128)
    wout_v = wout.rearrange("(ko p) c -> p ko c", p=128)

    with tile.TileContext(nc) as tc:
        with tc.tile_pool(name="persist", bufs=1) as persist, \
             tc.tile_pool(name="dram", bufs=1, space="DRAM") as dram_pool:
            masks_sb = persist.tile([128, 4, 512], BF16, name="masks")
            ones2_sb = persist.tile([128, 2, 128], FP8, name="ones2")
            onesb_sb = persist.tile([128, 128], BF16, name="onesb")
            bqk_sb = persist.tile([128, 4], F32, name="bqk")
            expb_sb = persist.tile([128, 1], F32, name="expb")
            bv_sb = persist.tile([128, 2 * HL * Dh], F32, name="bv")
            bout_sb = persist.tile([128, D], BF16, name="bout")
            w8qk_sb = persist.tile([128, KP, 2, CQK], FP8, name="w8qk")
            w8v_sb = persist.tile([128, KP, 2, CV], FP8, name="w8v")
            wqkvb_sb = [persist.tile([128, CQK + CV], BF16, name=f"wqkvb{ko}")
                        for ko in range(KO)]
            wout_sb = [persist.tile([128, D], BF16, name=f"wout{ko}")
                       for ko in range(KO)]
            # fp8 q/k for qc>=1 (large windows); bf16 q/k for chunk 0
            qT8 = [persist.tile([128, HL, T], FP8, name=f"qT8{b}") for b in range(B)]
            kT8 = [persist.tile([128, HL, T], FP8, name=f"kT8{b}") for b in range(B)]
            qTe = [persist.tile([128, HL, TC], BF16, name=f"qTe{b}") for b in range(B)]
            kTe = [persist.tile([128, HL, TC], BF16, name=f"kTe{b}") for b in range(B)]
            vv = [persist.tile([128, HL, NKB, Dh], FP8, name=f"v{b}")
                  for b in range(B)]
            vve = [persist.tile([128, HL, 4, Dh], BF16, name=f"ve{b}")
                   for b in range(B)]
            xb0 = persist.tile([128, KO, TC], BF16, name="xb0")

            nc.gpsimd.memset(expb_sb[:], EXPB)

            a2a_in = [[dram_pool.tile([W, HL * 128, TOKH], BF16, name=f"a2a_in{b}{h}")
                       for h in range(2)] for b in range(B)]
            a2a_out = [[dram_pool.tile([W, HL * 128, TOKH], BF16, name=f"a2a_out{b}{h}")
                        for h in range(2)] for b in range(B)]

            with tc.tile_pool(name="x8_pool", bufs=2) as x8_pool, \
                 tc.tile_pool(name="ex_pool", bufs=3) as ex_pool, \
                 tc.tile_pool(name="rden_pool", bufs=1) as rden_pool, \
                 tc.tile_pool(name="attc_pool", bufs=3) as attc_pool, \
                 tc.tile_pool(name="attall_pool", bufs=3) as attall_pool, \
                 tc.tile_pool(name="o_pool", bufs=3) as o_pool, \
                 tc.tile_pool(name="qk_psum", bufs=2, space="PSUM") as qk_psum, \
                 tc.tile_pool(name="s_psum", bufs=2, space="PSUM") as s_psum, \
                 tc.tile_pool(name="av_psum", bufs=1, space="PSUM") as av_psum, \
                 tc.tile_pool(name="den_psum", bufs=1, space="PSUM") as den_psum:

                def emit_proj_chunk8(b, ci, split=False):
                    """fp8 DoubleRow projection of one 512-token chunk (ci>=1)."""
                    t0 = b * T + ci * TC
                    x8_sb = x8_pool.tile([128, KP, 2, TC], FP8, name="x8_sb")
                    if split:
                        nc.sync.dma_start(x8_sb[:, 0:KP // 2],
                                          x8_v[:, 0:KP // 2, :, t0:t0 + TC])
                        nc.scalar.dma_start(x8_sb[:, KP // 2:],
                                            x8_v[:, KP // 2:, :, t0:t0 + TC])
                    else:
                        nc.sync.dma_start(x8_sb[:], x8_v[:, :, :, t0:t0 + TC])
                    for cc in range(4):             # 0,1: Q h0,h1; 2,3: K h0,h1
                        ps = qk_psum.tile([128, TC], F32, name="qk_ps", tag="qk")
                        for kp in range(KP):
                            nc.tensor.matmul(
                                ps[:],
                                w8qk_sb[:, kp, :, cc * 128:(cc + 1) * 128],
                                x8_sb[:, kp, :, :],
                                start=(kp == 0), stop=(kp == KP - 1),
                                perf_mode=DR)
                        dest = qT8[b] if cc < 2 else kT8[b]
                        nc.vector.tensor_scalar_add(
                            dest[:, cc % 2, ci * TC:(ci + 1) * TC], ps[:],
                            bqk_sb[:, cc:cc + 1])
                    for th in range(2):
                        psv = qk_psum.tile([128, 2, 256], F32, name="v_ps",
                                           tag="qk")
                        for tb in range(2):
                            tok = (th * 2 + tb) * 128
                            for kp in range(KP):
                                nc.tensor.matmul(
                                    psv[:, tb, :],
                                    x8_sb[:, kp, :, tok:tok + 128],
                                    w8v_sb[:, kp, :, :],
                                    start=(kp == 0), stop=(kp == KP - 1),
                                    perf_mode=DR)
                        vidx = ci * 4 + th * 2
                        nc.vector.tensor_tensor(
                            vv[b][:, :, vidx:vidx + 2, :],
                            psv[:].rearrange("p tb (hl d) -> p hl tb d", hl=HL),
                            bv_sb[:].rearrange("p (hl tb d) -> p hl tb d",
                                               hl=HL, tb=2),
                            ADD)

                def emit_proj_chunk_bf(b):
                    """bf16 projection of chunk 0 (small-window tokens).

                    Writes bf16 qTe/kTe/vve for the qc=0 attention group and
                    fp8 copies of k/v for later q-chunks' big-window reads."""
                    t0 = b * TC
                    nc.sync.dma_start(xb0[:, 0:KO // 2],
                                      xb_v[:, 0:KO // 2, t0:t0 + TC])
                    nc.scalar.dma_start(xb0[:, KO // 2:],
                                        xb_v[:, KO // 2:, t0:t0 + TC])
                    for cc in range(4):
                        ps = qk_psum.tile([128, TC], F32, name="qk_ps", tag="qk")
                        for ko in range(KO):
                            nc.tensor.matmul(
                                ps[:],
                                wqkvb_sb[ko][:, cc * 128:(cc + 1) * 128],
                                xb0[:, ko, :],
                                start=(ko == 0), stop=(ko == KO - 1))
                        dest = qTe[b] if cc < 2 else kTe[b]
                        nc.vector.tensor_scalar_add(
                            dest[:, cc % 2, :], ps[:], bqk_sb[:, cc:cc + 1])
                        if cc >= 2:     # fp8 copy of early k for qc>=1 groups
                            nc.vector.tensor_scalar_add(
                                kT8[b][:, cc % 2, 0:TC], ps[:],
                                bqk_sb[:, cc:cc + 1])
                    for th in range(2):
                        psv = qk_psum.tile([128, 2, 256], F32, name="v_ps",
                                           tag="qk")
                        for tb in range(2):
                            tok = (th * 2 + tb) * 128
                            for ko in range(KO):
                                nc.tensor.matmul(
                                    psv[:, tb, :],
                                    xb0[:, ko, tok:tok + 128],
                                    wqkvb_sb[ko][:, CQK:],
                                    start=(ko == 0), stop=(ko == KO - 1))
                        vidx = th * 2
                        bvr = bv_sb[:].rearrange("p (hl tb d) -> p hl tb d",
                                                 hl=HL, tb=2)
                        psr = psv[:].rearrange("p tb (hl d) -> p hl tb d", hl=HL)
                        nc.vector.tensor_tensor(
                            vve[b][:, :, vidx:vidx + 2, :], psr, bvr, ADD)
                        nc.vector.tensor_tensor(
                            vv[b][:, :, vidx:vidx + 2, :], psr, bvr, ADD)

                def emit_attn_group(b, hl, qc):
                    """One (head, q-chunk) group, all k-blocks as pair units.

                    qc>=1: S^T pair (fp8 q/k) -> batched exp -> fp8 ex2 ->
                    one DoubleRow PV + one DoubleRow ones-matmul (denom).
                    qc=0: same shape in bf16 (qTe/kTe/vve, bf16 ex2, plain
                    matmuls) because small windows don't average fp8 noise.
                    Diagonal blocks get full-width additive causal masks
                    before the exp, so masked entries exp to 0."""
                    bf = (qc == 0)
                    q0 = qc * QC
                    npu = (qc + 1) * 2          # pair units (256 keys each)
                    ps_av = av_psum.tile([128, QC], F32, name="ps_av")
                    ps_d = den_psum.tile([128, QC], F32, name="ps_d")
                    exs = {}

                    def emit_S(u):
                        kb = 2 * u
                        ps2 = s_psum.tile([128, 2, QC], F32, name="ps_s2")
                        for t in range(2):
                            if bf:
                                nc.tensor.matmul(
                                    ps2[:, t, :],
                                    kTe[b][:, hl, (kb + t) * 128:(kb + t + 1) * 128],
                                    qTe[b][:, hl, :], start=True, stop=True)
                            else:
                                nc.tensor.matmul(
                                    ps2[:, t, :],
                                    kT8[b][:, hl, (kb + t) * 128:(kb + t + 1) * 128],
                                    qT8[b][:, hl, q0:q0 + QC],
                                    start=True, stop=True)
                            dv = kb + t - qc * 4
                            if dv >= 0:
                                nc.vector.tensor_tensor(
                                    ps2[:, t, :], ps2[:, t, :],
                                    masks_sb[:, dv, :], ADD)
                        ex2 = ex_pool.tile([128, 2, QC], BF16 if bf else FP8,
                                           name="ex2", tag="ex2")
                        nc.scalar.activation(
                            ex2[:], ps2[:],
                            mybir.ActivationFunctionType.Exp,
                            scale=SCALE, bias=expb_sb[:])
                        exs[u] = ex2

                    def emit_PV(u, first, last):
                        kb = 2 * u
                        ex2 = exs.pop(u)
                        if bf:
                            for t in range(2):
                                nc.tensor.matmul(
                                    ps_av[:], vve[b][:, hl, kb + t, :],
                                    ex2[:, t, :],
                                    start=first and t == 0, stop=last and t == 1)
                                nc.tensor.matmul(
                                    ps_d[:], onesb_sb[:], ex2[:, t, :],
                                    start=first and t == 0, stop=last and t == 1)
                        else:
                            nc.tensor.matmul(
                                ps_av[:], vv[b][:, hl, kb:kb + 2, :], ex2[:],
                                start=first, stop=last, perf_mode=DR)
                            nc.tensor.matmul(
                                ps_d[:], ones2_sb[:], ex2[:],
                                start=first, stop=last, perf_mode=DR)

                    emit_S(0)
                    for j in range(1, npu):
                        emit_S(j)
                        emit_PV(j - 1, j == 1, False)
                    emit_PV(npu - 1, npu == 1, True)

                    rden = rden_pool.tile([128, QC], F32, name="rden")
                    nc.vector.reciprocal_approx_fast(rden[:], ps_d[:])
                    attc = attc_pool.tile([128, QC], BF16, name="attc")
                    nc.vector.tensor_tensor(attc[:], ps_av[:], rden[:], MULT)
                    h = qc // 2
                    view = a2a_in[b][h].rearrange(
                        "(hh rr) (hl p) t -> p hl hh rr t",
                        hh=2, rr=W // 2, hl=HL, p=128)
                    nc.gpsimd.dma_start(
                        view[:, hl, qc % 2],
                        attc[:].rearrange("p (rr t) -> p rr t", rr=W // 2))

                def emit_a2a(b, h):
                    nc.gpsimd.collective_compute(
                        "AllToAll", mybir.AluOpType.bypass,
                        replica_groups=[list(range(W))],
                        ins=[a2a_in[b][h][:].opt()], outs=[a2a_out[b][h][:].opt()])

                def emit_attall(b, h, slot):
                    ga = attall_pool.tile([128, KO, TOKH], BF16, name="attall")
                    nc.scalar.dma_start(
                        ga[:],
                        a2a_out[b][h].rearrange("r (hl p) t -> p (r hl) t",
                                                hl=HL, p=128))
                    slot[(b, h)] = ga

                def emit_outproj(b, h, slot):
                    ga = slot[(b, h)]
                    for colc in range(D // 512):
                        ps_o = qk_psum.tile([128, 512], F32, name="qk_ps",
                                            tag="qk")
                        for ko in range(KO):
                            nc.tensor.matmul(
                                ps_o[:], ga[:, ko, :],
                                wout_sb[ko][:, colc * 512:(colc + 1) * 512],
                                start=(ko == 0), stop=(ko == KO - 1))
                        o_sb = o_pool.tile([128, 512], F32, name="o_sb")
                        nc.vector.tensor_tensor(
                            o_sb[:], ps_o[:],
                            bout_sb[:, colc * 512:(colc + 1) * 512], ADD)
                        nc.sync.dma_start(
                            out[(b * 2 + h) * TOKH:(b * 2 + h + 1) * TOKH,
                                colc * 512:(colc + 1) * 512],
                            o_sb[:])

                ga = {}
                # startup: fp8 weights first on scalar (the first chunk's
                # matmuls need them), x8 chunk on sync; the 3MB bf16
                # chunk-0 weights stream on gpsimd (not needed until ~60us
                # in); wout (8.5MB) is gated behind proj progress below.
                nc.scalar.dma_start(
                    w8qk_sb[:],
                    w8qk.rearrange("p (kp two c) -> p kp two c", kp=KP, two=2))
                nc.scalar.dma_start(
                    w8v_sb[:],
                    w8vd.rearrange("p (kp two c) -> p kp two c", kp=KP, two=2))
                emit_proj_chunk8(0, 1)
                nc.sync.dma_start(bqk_sb[:], bqkd)
                nc.sync.dma_start(
                    ones2_sb[:], ones2d.rearrange("p (two q) -> p two q", two=2))
                nc.sync.dma_start(onesb_sb[:], onesbd)
                nc.sync.dma_start(bv_sb[:], bvbc)
                # gate the 3.5MB of bf16 weights + masks behind chunk-1
                # progress so they can't steal startup HBM bandwidth from
                # the critical x8/w8qk transfers
                gate0 = qT8[0][:, 0, TC + 1:TC + 2]
                nc.vector.tensor_scalar_add(masks_sb[:, 0, 0:1], gate0, 0.0)
                nc.sync.dma_start(
                    masks_sb[:], masks4.rearrange("p (dv q) -> p dv q", dv=4))
                for ko in range(KO):
                    nc.vector.tensor_scalar_add(
                        wqkvb_sb[ko][:, 0:1], gate0, 0.0)
                    nc.gpsimd.dma_start(wqkvb_sb[ko][:], wqkvb_v[:, ko, :])

                def emit_wout_load(step):
                    # Gate each wout tile behind batch-1 proj progress (WAW
                    # on the DMA): wout is only needed by the out-projections
                    # at the tail, and these 8.5MB would otherwise clog the
                    # sync queue that carries the x8 chunk prefetches.
                    gate = (qTe[1][:, 0, 1:2] if step == 3
                            else qT8[1][:, 0, (step + 1) * TC + 1:(step + 1) * TC + 2])
                    if step == 0:
                        nc.vector.tensor_scalar_add(bout_sb[:, 0:1], gate, 0.0)
                        nc.sync.dma_start(bout_sb[:], boutbc)
                    for ko in range(4 * step, 4 * step + 4):
                        nc.vector.tensor_scalar_add(
                            wout_sb[ko][:, 0:1], gate, 0.0)
                        nc.sync.dma_start(wout_sb[ko][:], wout_v[:, ko, :])

                emit_proj_chunk8(0, 2)
                emit_proj_chunk8(0, 3)
                emit_proj_chunk_bf(0)
                # batch-0 attention interleaved with batch-1 projection
                emit_attn_group(0, 0, 0)
                emit_attn_group(0, 1, 0)
                emit_proj_chunk8(1, 1)
                emit_wout_load(0)
                emit_attn_group(0, 0, 1)
                emit_attn_group(0, 1, 1)
                emit_a2a(0, 0)
                emit_proj_chunk8(1, 2)
                emit_wout_load(1)
                emit_attn_group(0, 0, 2)
                emit_attn_group(0, 1, 2)
                emit_proj_chunk8(1, 3)
                emit_wout_load(2)
                emit_attn_group(0, 0, 3)
                emit_attn_group(0, 1, 3)
                emit_a2a(0, 1)
                emit_proj_chunk_bf(1)
                emit_wout_load(3)
                # batch-1 attention; batch-0 out-projections fill scalar gaps
                emit_attn_group(1, 0, 0)
                emit_attn_group(1, 1, 0)
                emit_attall(0, 0, ga)
                emit_attn_group(1, 0, 1)
                emit_attn_group(1, 1, 1)
                emit_a2a(1, 0)
                emit_attall(0, 1, ga)
                emit_attn_group(1, 0, 2)
                emit_attn_group(1, 1, 2)
                emit_attall(1, 0, ga)
                emit_outproj(0, 0, ga)
                emit_attn_group(1, 0, 3)
                emit_attn_group(1, 1, 3)
                emit_a2a(1, 1)
                emit_outproj(0, 1, ga)
                emit_outproj(1, 0, ga)
                emit_attall(1, 1, ga)
                emit_outproj(1, 1, ga)
    nc.compile()
    return nc


_CACHED_NC = None


def kernel(x, Wqkv, bqkv, Wout, bout):
    global _CACHED_NC
    x = np.asarray(x, dtype=np.float32)
    Wqkv = np.asarray(Wqkv, dtype=np.float32)
    bqkv = np.asarray(bqkv, dtype=np.float32)
    Wout = np.asarray(Wout, dtype=np.float32)
    bout = np.asarray(bout, dtype=np.float32)

    if _CACHED_NC is None:
        _CACHED_NC = _build()
    nc = _CACHED_NC

    bf16 = ml_dtypes.bfloat16
    fp8 = ml_dtypes.float8_e4m3fn
    xTf = np.ascontiguousarray(x.reshape(NT, D).T)              # [D, NT] f32
    x8 = xTf.astype(fp8)
    xb = np.ascontiguousarray(
        np.concatenate([xTf[:, b * T:b * T + TC] for b in range(B)], axis=1)
    ).astype(bf16)                                              # [D, B*TC]
    wq4 = Wqkv.reshape(D, 3, H, Dh)                 # col = (which, head, dh)
    bq4 = bqkv.reshape(3, H, Dh)
    kl = np.arange(128)[:, None]
    ql = np.arange(512)[None, :]
    masks4 = np.stack(
        [np.where(ql >= dv * 128 + kl, 0.0, -1e9) for dv in range(4)],
        axis=1).reshape(128, 4 * 512).astype(bf16)
    ones2 = np.ones((128, 256), dtype=fp8)
    onesb = np.ones((128, 128), dtype=bf16)
    wout_bf = Wout.astype(bf16)
    boutbc = np.tile(bout[None, :], (128, 1)).astype(bf16)

    def dr_pack(w):
        # [D, C] -> [128, (kp two c)] matching the DoubleRow stationary AP
        cdim = w.shape[1]
        return np.ascontiguousarray(
            w.reshape(KP, 2, 128, cdim).transpose(2, 0, 1, 3)
            .reshape(128, KP * 2 * cdim))

    in_maps = []
    for c in range(W):
        wqk = np.ascontiguousarray(
            wq4[:, 0:2, HL * c:HL * c + HL, :].reshape(D, CQK))
        wv = np.ascontiguousarray(
            wq4[:, 2, HL * c:HL * c + HL, :].reshape(D, CV))
        wfull = np.ascontiguousarray(
            np.concatenate([wqk, wv], axis=1)).astype(bf16)     # [D, 768]
        bshard_qk = np.ascontiguousarray(
            bq4[0:2, HL * c:HL * c + HL, :].reshape(CQK)).astype(np.float32)
        bqk_pc = np.ascontiguousarray(
            bshard_qk.reshape(4, 128).T).astype(np.float32)     # [128, 4]
        bshard_v = bq4[2, HL * c:HL * c + HL, :]                  # [HL, Dh]
        bvbc = np.ascontiguousarray(np.broadcast_to(
            bshard_v.reshape(1, HL, 1, Dh), (128, HL, 2, Dh)
        ).reshape(128, 2 * HL * Dh)).astype(np.float32)
        in_maps.append({
            "x8d": x8, "xbd": xb,
            "w8qk": dr_pack(wqk).astype(fp8),
            "w8vd": dr_pack(wv).astype(fp8),
            "wqkvb": wfull, "bqkd": bqk_pc, "wout": wout_bf,
            "masks4": masks4, "ones2d": ones2, "onesbd": onesb,
            "bvbc": bvbc, "boutbc": boutbc,
        })

    res = run_bass_kernel_spmd(nc, in_maps, core_ids=list(range(W)))
    # res[c]["out"] rows [(b*2+h)*TOKH ...) = tokens [h*HT + c*TOKH ...) of batch b
    full = np.empty((B, T, D), np.float32)
    for c in range(W):
        for b in range(B):
            for h in range(2):
                full[b, h * HT + c * TOKH:h * HT + (c + 1) * TOKH] = \
                    res.results[c]["out"][(b * 2 + h) * TOKH:(b * 2 + h + 1) * TOKH]
    return full
